# revision 26
# baseline (speedup 1.0000x reference)
"""Trainium2 Bass kernel for nn_CSMv2Agent (CSMv2 agent step), 8-core data parallel.

Host side: packs inputs into one fused [B, 431] fp32 tensor + packed fp16
parameter tensors, shards batch across 8 NeuronCores, runs one SPMD NEFF,
unpacks the fused [B, 584] output into the reference's 14-tuple.

Device side (per core, B_c = 4096, 8 mega-tiles of 512 samples):
  - fp16 on-chip compute (PSUM accumulation fp32), feature-major lhsT tiles
    produced by PE transposes, batch-major heads, identity-matmul PSUM
    accumulation for the 16-expert routing reduction.
"""

import numpy as np

import concourse.bass as bass
import concourse.tile as tile
from concourse import bacc, mybir
from concourse.bass_utils import run_bass_kernel_spmd

F32 = mybir.dt.float32
F16 = mybir.dt.float16
AF = mybir.ActivationFunctionType
OP = mybir.AluOpType
AX = mybir.AxisListType

# Problem constants
B, OBS, H, NA = 32768, 56, 64, 6
NSC, TOPK, SD, MD, GD, NSLOT, NVAR = 16, 2, 32, 16, 8, 4, 4
NCORES = 8
BC = B // NCORES            # 4096 per core
MEGA = 512                  # samples per mega-tile
NMEGA = BC // MEGA          # 8
NSUB = MEGA // 128          # 4

# Fused input columns: obs | prev_h | pa | S_obj | S_meta | slots | ext
IC_OBS, IC_PH, IC_PA = 0, 56, 120
IC_SO, IC_SM, IC_SL, IC_EXT = 126, 158, 174, 430
IN_W = 431

# Fused output columns
OC_AP, OC_V, OC_GOAL, OC_SW = 0, 6, 7, 15
OC_IP, OC_VAR, OC_C0, OC_C1, OC_C2 = 31, 34, 38, 54, 86
OC_H, OC_Y, OC_SON, OC_SMN, OC_SLN = 150, 214, 278, 310, 326
OUT_W = 584  # 582 used + 2 pad


def _f16(x):
    return np.ascontiguousarray(np.asarray(x), dtype=np.float16)


def _f32(x):
    return np.ascontiguousarray(np.asarray(x), dtype=np.float32)


def prep_params(p):
    """Host-side packing of the parameter pytree into device tensors."""
    t = {}
    t["W_enc"] = _f16(p["enc_W"])                       # [56,64]
    t["b_enc"] = _f32(np.asarray(p["enc_b"]).reshape(64, 1))
    t["W_wm"] = _f16(p["wm_W"])                         # [70,64]
    t["b_wm"] = _f16(np.asarray(p["wm_b"]).reshape(1, 64))
    t["W_mg"] = _f16(np.asarray(p["mg_W"])[0:64])       # [64,8] h part
    t["b_mg"] = _f16(np.asarray(p["mg_b"]).reshape(1, 8))
    t["Wcap"] = _f32(np.asarray(p["mg_W"])[65])         # [8] fp32: logits path
    t["Wext"] = _f32(np.asarray(p["mg_W"])[66])         # [8] fp32: logits path
    t["W_rt"] = _f16(np.asarray(p["rt_W"])[0:64])       # [64,16]
    t["W_rtg"] = _f16(np.asarray(p["rt_W"])[64:72])     # [8,16]
    t["b_rt"] = _f16(np.asarray(p["rt_b"]).reshape(1, 16))
    exw = np.asarray(p["ex_W"])                          # [16,64,64]
    exb = np.asarray(p["ex_b"])                          # [16,64]
    wex = np.concatenate(
        [exw.transpose(1, 0, 2).reshape(64, NSC * 64),
         exb.reshape(1, NSC * 64)], axis=0)              # [65,1024]
    t["W_ex"] = _f16(wex)
    t["W_mc"] = _f16(np.concatenate(
        [np.asarray(p["mc_in_W"])[0:32], np.asarray(p["mc_rec_W"])], axis=0))  # [48,16]
    t["b_mc"] = _f16(np.asarray(p["mc_in_b"]).reshape(1, 16))
    t["Wpe"] = _f16(np.asarray(p["mc_in_W"])[32] / 64.0)  # [16] /H folded
    t["Wconf"] = _f16(-np.asarray(p["mc_in_W"])[33])     # [16] negated
    t["Went"] = _f16(-np.asarray(p["mc_in_W"])[34])      # [16] negated
    t["W_dAip"] = _f16(np.concatenate(
        [np.asarray(p["dA_W"]), np.asarray(p["ip_W"])], axis=1))  # [16,4]
    t["b_dAip"] = _f16(np.concatenate(
        [np.asarray(p["dA_b"]), np.asarray(p["ip_b"])]).reshape(1, 4))
    t["negspA"] = _f16(-np.log1p(np.exp(np.asarray(p["A"], np.float64))))  # [32]
    t["W_mbqv"] = _f16(np.concatenate(
        [np.asarray(p["mB_W"]), np.asarray(p["Wq"]) / np.sqrt(H),
         np.asarray(p["Wv"])], axis=1))                  # [64,160]
    t["b_mbqv"] = _f16(np.concatenate(
        [np.asarray(p["mB_b"]), np.zeros(128, np.float32)]).reshape(1, 160))
    t["W_yc1"] = _f16(np.concatenate(
        [np.asarray(p["mC_W"]), np.asarray(p["c1_W"])], axis=1))  # [32,96]
    t["b_yc1"] = _f16(np.concatenate(
        [np.asarray(p["mC_b"]), np.asarray(p["c1_b"])]).reshape(1, 96))
    t["W_c2a"] = _f16(np.asarray(p["c2_W"])[0:32])       # [32,64]
    t["W_c2b"] = _f16(np.asarray(p["c2_W"])[32:96])      # [64,64]
    t["b_c2"] = _f16(np.asarray(p["c2_b"]).reshape(1, 64))
    t["W_piv"] = _f16(np.concatenate(
        [np.asarray(p["pi_W"]), np.asarray(p["v_W"])], axis=1))   # [64,7]
    t["b_piv"] = _f16(np.concatenate(
        [np.asarray(p["pi_b"]), np.asarray(p["v_b"])]).reshape(1, 7))
    t["W_c0cg"] = _f16(np.concatenate(
        [np.asarray(p["c0_W"]), np.asarray(p["cg_W"])], axis=1))  # [64,20]
    t["b_c0cg"] = _f16(np.concatenate(
        [np.asarray(p["c0_b"]), np.asarray(p["cg_b"])]).reshape(1, 20))
    def _res(w):
        w = np.asarray(w, np.float32)
        return _f16(w.astype(np.float64) - w.astype(np.float16).astype(np.float64))
    t["dW_enc"] = _res(p["enc_W"])
    t["dW_mg"] = _res(np.asarray(p["mg_W"])[0:64])
    t["dW_rt"] = _res(np.asarray(p["rt_W"])[0:64])
    t["dW_rtg"] = _res(np.asarray(p["rt_W"])[64:72])
    t["db_mg"] = _res(np.asarray(p["mg_b"]).reshape(1, 8))
    t["db_rt"] = _res(np.asarray(p["rt_b"]).reshape(1, 16))
    t["ident"] = _f16(np.eye(128))                       # [128,128]
    t["ones"] = _f16(np.ones((1, MEGA)))                 # [1,512]
    return t


PARAM_SPECS = None  # filled by prep (name -> (shape, dtype))


def _fv(ap2d, dims):
    """Replace the free dims of a (partition, free...) AP with explicit
    [step, count] pairs (element units). Keeps partition dim + offset."""
    return bass.AP(tensor=ap2d.tensor, offset=ap2d.offset,
                   ap=[list(ap2d.ap[0])] + [list(d) for d in dims])


def build_kernel(param_specs):
    nc = bacc.Bacc("TRN2", target_bir_lowering=False, debug=False,
                   num_devices=NCORES)

    dX = nc.dram_tensor("X", [BC, IN_W], F32, kind="ExternalInput").ap()
    dE = nc.dram_tensor("EXT", [BC], F32, kind="ExternalInput").ap()
    dO = nc.dram_tensor("OUT", [BC, OUT_W], F32, kind="ExternalOutput").ap()
    dP = {}
    for name, (shape, dt) in param_specs.items():
        dP[name] = nc.dram_tensor("p_" + name, list(shape), dt,
                                  kind="ExternalInput").ap()

    with tile.TileContext(nc) as tc:
        _body(nc, tc, dX, dE, dO, dP)
    nc.compile()
    return nc


def _body(nc, tc, dX, dE, dO, dP):
    import contextlib
    ctx = contextlib.ExitStack()
    cp = ctx.enter_context(tc.tile_pool(name="const", bufs=1))
    io = ctx.enter_context(tc.tile_pool(name="io", bufs=2))
    sb = ctx.enter_context(tc.tile_pool(name="work", bufs=2))
    big = ctx.enter_context(tc.tile_pool(name="big", bufs=2))
    psA = ctx.enter_context(tc.tile_pool(name="psA", bufs=2, space="PSUM"))
    psB = ctx.enter_context(tc.tile_pool(name="psB", bufs=4, space="PSUM"))

    # ---- load params once ----
    P = {}
    for name, ap in dP.items():
        shp = list(ap.shape)
        if len(shp) == 1:
            # row-vector params broadcast to [128, n] tiles
            tl = cp.tile([128, shp[0]], ap.dtype, tag="par_" + name)
            nc.sync.dma_start(tl, bass.AP(tensor=ap.tensor, offset=0,
                                          ap=[[0, 128], [1, shp[0]]]))
        else:
            tl = cp.tile(shp, ap.dtype, tag="par_" + name)
            nc.sync.dma_start(tl, ap)
        P[name] = tl
    I16 = P["ident"]
    onesr = cp.tile([1, MEGA], F16)
    nc.sync.dma_start(onesr, dP["ones"])
    eps8 = cp.tile([128, 1], F32)
    nc.vector.memset(eps8, 1e-8)
    i32 = cp.tile([1, 1], F32)
    nc.vector.memset(i32, 1.0)
    cp_i32full = cp.tile([128, 128], F32)
    nc.gpsimd.memset(cp_i32full[:], 0.0)
    from concourse.masks import make_identity as _mkid
    _mkid(nc, cp_i32full[:], nomemset=True)

    Xv = dX.rearrange("(m j p) c -> m p j c", p=128, j=NSUB)      # [8,128,4,431]
    Ev = dE.rearrange("(m t) -> m t", t=MEGA)                     # [8,512]
    Ov = dO.rearrange("(m j p) c -> m p j c", p=128, j=NSUB)      # [8,128,4,584]

    import os as _os
    _nm = int(_os.environ.get("KERNEL_NMEGA", NMEGA))
    _SEC = int(_os.environ.get("KERNEL_SEC", 99))
    for m in range(_nm):
        # ================= loads =================
        stag = io.tile([128, NSUB, IN_W], F16, tag="stag")
        nc.gpsimd.dma_start(stag[:], Xv[m])       # cast f32 -> f16
        extr = io.tile([1, MEGA], F32, tag="extr")
        nc.sync.dma_start(extr[:], Ev[m][None, :])
        stage = io.tile([128, NSUB, OUT_W], F16, tag="stage")
        if _SEC < 99:
            nc.vector.memset(stage[:], 0.0)

        # ================= input transposes =================
        obs32 = io.tile([128, NSUB, OBS], F32, tag="obs32")
        nc.sync.dma_start(obs32[:], Xv[m][:, :, 0:OBS])
        i32f = cp_i32full
        t1ps = psB.tile([56, MEGA], F32, tag="pss")
        t1bps = psB.tile([70, MEGA], F16, tag="pss")
        t2ps = psB.tile([48, MEGA], F16, tag="pss")
        for j in range(NSUB):
            nc.tensor.transpose(t1ps[:, bass.ts(j, 128)],
                                obs32[:, j, :], i32f[:])
            nc.tensor.transpose(t1bps[:, bass.ts(j, 128)],
                                stag[:, j, 56:126], I16[:])
            nc.tensor.transpose(t2ps[:, bass.ts(j, 128)],
                                stag[:, j, 126:174], I16[:])
        obsT = big.tile([56, MEGA], F16, tag="obsT")
        nc.scalar.copy(obsT[:], t1ps[:])
        dobsT = big.tile([56, MEGA], F16, tag="dobsT")
        nc.vector.tensor_tensor(dobsT[:], t1ps[:], obsT[:], op=OP.subtract)
        phpaT = big.tile([70, MEGA], F16, tag="phpaT")
        nc.scalar.copy(phpaT[:], t1bps[:])
        xT2 = big.tile([48, MEGA], F16, tag="xT2")
        nc.vector.tensor_copy(xT2[:], t2ps[:])

        if _SEC < 1:
            nc.gpsimd.dma_start(Ov[m][:, :, 0:582], stage[:, :, 0:582])
            continue
        # ================= encoder (feature-major) =================
        hTps = psB.tile([64, MEGA], F32, tag="pss")
        nc.tensor.matmul(hTps[:], P["W_enc"][:], obsT[:],
                         start=True, stop=False)
        nc.tensor.matmul(hTps[:], P["W_enc"][:], dobsT[:],
                         start=False, stop=False)
        nc.tensor.matmul(hTps[:], P["dW_enc"][:], obsT[:],
                         start=False, stop=True)
        hT32 = big.tile([64, MEGA], F32, tag="hT32")
        nc.scalar.activation(hT32[:], hTps[:], AF.Tanh,
                             bias=P["b_enc"][:], scale=1.0)
        stk = big.tile([65, MEGA], F16, tag="stk")   # rows 0:64 hT, 64 ones
        nc.vector.tensor_copy(stk[0:64, :], hT32[:])
        dhT = big.tile([64, MEGA], F16, tag="dhT")
        nc.vector.tensor_tensor(dhT[:], hT32[:], stk[0:64, :], op=OP.subtract)
        nc.sync.dma_start(stk[64:65, :], dP["ones"])

        if _SEC < 2:
            nc.gpsimd.dma_start(Ov[m][:, :, 0:582], stage[:, :, 0:582])
            continue
        # ================= h (batch-major) + world model =================
        hbps = psB.tile([128, NSUB, H], F16, tag="pss")
        for j in range(NSUB):
            nc.tensor.transpose(hbps[:, j, :], stk[0:64, bass.ts(j, 128)],
                                I16[0:64, 0:64])
        nc.scalar.copy(stage[:, :, OC_H:OC_H + H], hbps[:])

        predps = psB.tile([128, NSUB, H], F32, tag="pss")
        for j in range(NSUB):
            nc.tensor.matmul(predps[:, j, :], phpaT[:, bass.ts(j, 128)],
                             P["W_wm"][:], start=True, stop=False)
            nc.tensor.matmul(predps[:, j, :], onesr[:, bass.ts(j, 128)],
                             P["b_wm"][0:1, :], start=False, stop=True)
        diff = sb.tile([128, NSUB, H], F16, tag="diff")
        nc.vector.tensor_tensor(diff[:], predps[:],
                                stage[:, :, OC_H:OC_H + H],
                                op=OP.subtract)
        dsq = sb.tile([128, NSUB, H], F16, tag="dsq")
        nc.vector.tensor_tensor(dsq[:], diff[:], diff[:], op=OP.mult)
        pe = sb.tile([128, NSUB], F32, tag="pe")
        nc.vector.tensor_reduce(pe[:], dsq[:], axis=AX.X, op=OP.add)
        # pe here is SUM of squares; /H folded into Wpe (host) and conf below
        # conf'' = min(pe/H,1) - 1  (W row negated on host)
        confa = sb.tile([128, NSUB], F32, tag="confa")
        nc.vector.tensor_scalar(confa[:], pe[:], 1.0 / H, 1.0,
                                op0=OP.mult, op1=OP.min)
        conf = sb.tile([128, NSUB], F32, tag="conf")
        nc.vector.tensor_scalar(conf[:], confa[:], 1.0, None,
                                op0=OP.subtract)

        if _SEC < 3:
            nc.gpsimd.dma_start(Ov[m][:, :, 0:582], stage[:, :, 0:582])
            continue
        # ================= ext / capability gap =================
        extps = psB.tile([128, NSUB], F32, tag="pss")
        for j in range(NSUB):
            nc.tensor.transpose(extps[:, j:j + 1],
                                extr[0:1, bass.ts(j, 128)], i32[:])
        ext16 = sb.tile([128, NSUB], F32, tag="ext16")
        nc.scalar.copy(ext16[:], extps[:])
        capa = sb.tile([128, NSUB], F32, tag="capa")
        nc.vector.tensor_scalar(capa[:], extps[:], 0.0, 0.1,
                                op0=OP.is_ge, op1=OP.mult)
        capb = sb.tile([128, NSUB], F32, tag="capb")
        nc.scalar.activation(capb[:], extps[:], AF.Relu, bias=0.0, scale=-1.0)
        cap16 = sb.tile([128, NSUB], F32, tag="cap16")
        nc.vector.tensor_tensor(cap16[:], capa[:], capb[:], op=OP.add)

        if _SEC < 4:
            nc.gpsimd.dma_start(Ov[m][:, :, 0:582], stage[:, :, 0:582])
            continue
        # ================= meta goal =================
        goalps = psB.tile([128, NSUB, GD], F32, tag="pss")
        for j in range(NSUB):
            nc.tensor.matmul(goalps[:, j, :], stk[0:64, bass.ts(j, 128)],
                             P["W_mg"][:], start=True, stop=False)
            nc.tensor.matmul(goalps[:, j, :], dhT[:, bass.ts(j, 128)],
                             P["W_mg"][:], start=False, stop=False)
            nc.tensor.matmul(goalps[:, j, :], stk[0:64, bass.ts(j, 128)],
                             P["dW_mg"][:], start=False, stop=False)
            nc.tensor.matmul(goalps[:, j, :], onesr[:, bass.ts(j, 128)],
                             P["b_mg"][0:1, :], start=False, stop=False)
            nc.tensor.matmul(goalps[:, j, :], onesr[:, bass.ts(j, 128)],
                             P["db_mg"][0:1, :], start=False, stop=True)
        tt1 = sb.tile([128, NSUB, GD], F32, tag="gtt1")
        nc.vector.tensor_tensor(
            tt1[:], _fv(P["Wcap"][:], [[0, NSUB], [1, GD]]),
            _fv(cap16[:], [[1, NSUB], [0, GD]]), op=OP.mult)
        tt2 = sb.tile([128, NSUB, GD], F32, tag="gtt2")
        nc.vector.tensor_tensor(
            tt2[:], _fv(P["Wext"][:], [[0, NSUB], [1, GD]]),
            _fv(ext16[:], [[1, NSUB], [0, GD]]), op=OP.mult)
        gg1 = sb.tile([128, NSUB, GD], F32, tag="gg1")
        nc.vector.tensor_tensor(gg1[:], goalps[:], tt1[:], op=OP.add)
        gg2 = sb.tile([128, NSUB, GD], F32, tag="gg2")
        nc.vector.tensor_tensor(gg2[:], gg1[:], tt2[:], op=OP.add)
        goal32 = sb.tile([128, NSUB, GD], F32, tag="goal32")
        nc.scalar.activation(goal32[:], gg2[:], AF.Tanh)
        goal16 = sb.tile([128, NSUB, GD], F16, tag="goal16")
        nc.vector.tensor_copy(goal16[:], goal32[:])
        dgoal = sb.tile([128, NSUB, GD], F16, tag="dgoal")
        nc.vector.tensor_tensor(dgoal[:], goal32[:], goal16[:],
                                op=OP.subtract)
        nc.scalar.copy(stage[:, :, OC_GOAL:OC_GOAL + GD], goal32[:])
        gTps = psB.tile([GD, MEGA], F16, tag="pss")
        gdTps = psB.tile([GD, MEGA], F16, tag="pss")
        for j in range(NSUB):
            nc.tensor.transpose(gTps[:, bass.ts(j, 128)], goal16[:, j, :],
                                I16[:])
            nc.tensor.transpose(gdTps[:, bass.ts(j, 128)], dgoal[:, j, :],
                                I16[:])
        goalT = sb.tile([GD, MEGA], F16, tag="goalT")
        nc.vector.tensor_copy(goalT[:], gTps[:])
        dgoalT = sb.tile([GD, MEGA], F16, tag="dgoalT")
        nc.scalar.copy(dgoalT[:], gdTps[:])

        if _SEC < 5:
            nc.gpsimd.dma_start(Ov[m][:, :, 0:582], stage[:, :, 0:582])
            continue
        # ================= router logits + top-2 =================
        lps = psB.tile([128, NSUB, NSC], F32, tag="pss")
        for j in range(NSUB):
            nc.tensor.matmul(lps[:, j, :], stk[0:64, bass.ts(j, 128)],
                             P["W_rt"][:], start=True, stop=False)
            nc.tensor.matmul(lps[:, j, :], dhT[:, bass.ts(j, 128)],
                             P["W_rt"][:], start=False, stop=False)
            nc.tensor.matmul(lps[:, j, :], stk[0:64, bass.ts(j, 128)],
                             P["dW_rt"][:], start=False, stop=False)
            nc.tensor.matmul(lps[:, j, :], goalT[:, bass.ts(j, 128)],
                             P["W_rtg"][:], start=False, stop=False)
            nc.tensor.matmul(lps[:, j, :], dgoalT[:, bass.ts(j, 128)],
                             P["W_rtg"][:], start=False, stop=False)
            nc.tensor.matmul(lps[:, j, :], goalT[:, bass.ts(j, 128)],
                             P["dW_rtg"][:], start=False, stop=False)
            nc.tensor.matmul(lps[:, j, :], onesr[:, bass.ts(j, 128)],
                             P["b_rt"][0:1, :], start=False, stop=False)
            nc.tensor.matmul(lps[:, j, :], onesr[:, bass.ts(j, 128)],
                             P["db_rt"][0:1, :], start=False, stop=True)
        L0 = sb.tile([128, NSUB, NSC], F32, tag="L0")
        nc.vector.tensor_copy(L0[:], lps[:])
        mx = sb.tile([128, NSUB, 8], F32, tag="mx")
        for j in range(NSUB):
            nc.vector.max(mx[:, j, :], L0[:, j, :])
        rep = sb.tile([128, NSUB, 8], F32, tag="rep")
        nc.vector.memset(rep[:], 1e30)
        nc.vector.tensor_copy(rep[:, :, 0:1], mx[:, :, 0:1])
        L1 = sb.tile([128, NSUB, NSC], F32, tag="L1")
        for j in range(NSUB):
            nc.vector.match_replace(L1[:, j, :], rep[:, j, :], L0[:, j, :],
                                    -1e30)
        rep2 = sb.tile([128, NSUB, 8], F32, tag="rep2")
        nc.vector.memset(rep2[:], 1e30)
        nc.vector.tensor_copy(rep2[:, :, 0:1], mx[:, :, 1:2])
        L2 = sb.tile([128, NSUB, NSC], F32, tag="L2")
        for j in range(NSUB):
            nc.vector.match_replace(L2[:, j, :], rep2[:, j, :], L1[:, j, :],
                                    -1e30)
        m1 = sb.tile([128, NSUB, NSC], F16, tag="m1")
        nc.vector.tensor_tensor(m1[:], L0[:], L1[:], op=OP.not_equal)
        m2 = sb.tile([128, NSUB, NSC], F16, tag="m2")
        nc.vector.tensor_tensor(m2[:], L1[:], L2[:], op=OP.not_equal)
        dv = sb.tile([128, NSUB], F32, tag="dv")
        nc.vector.tensor_tensor(dv[:], mx[:, :, 1], mx[:, :, 0],
                                op=OP.subtract)
        ew = sb.tile([128, NSUB], F32, tag="ew")
        nc.scalar.activation(ew[:], dv[:], AF.Exp)
        zw = sb.tile([128, NSUB], F32, tag="zw")
        nc.vector.tensor_scalar_add(zw[:], ew[:], 1.0)
        w1 = sb.tile([128, NSUB], F32, tag="w1")
        nc.vector.reciprocal(w1[:], zw[:])
        w2 = sb.tile([128, NSUB], F32, tag="w2")
        nc.vector.tensor_scalar(w2[:], w1[:], -1.0, 1.0,
                                op0=OP.mult, op1=OP.add)
        swst = stage[:, :, OC_SW:OC_SW + NSC]
        nc.vector.tensor_tensor(swst, m1[:],
                                _fv(w1[:], [[1, NSUB], [0, NSC]]),
                                op=OP.mult)
        sw2t = sb.tile([128, NSUB, NSC], F16, tag="sw2t")
        nc.vector.tensor_tensor(sw2t[:], m2[:],
                                _fv(w2[:], [[1, NSUB], [0, NSC]]),
                                op=OP.mult)
        nc.vector.tensor_tensor(swst, swst, sw2t[:], op=OP.add)
        # entropy (negated): w1*ln(w1+eps) + w2*ln(w2+eps)
        lw1 = sb.tile([128, NSUB], F32, tag="lw1")
        nc.scalar.activation(lw1[:], w1[:], AF.Ln, bias=eps8[:])
        lw2 = sb.tile([128, NSUB], F32, tag="lw2")
        nc.scalar.activation(lw2[:], w2[:], AF.Ln, bias=eps8[:])
        en1 = sb.tile([128, NSUB], F32, tag="en1")
        nc.vector.tensor_tensor(en1[:], w1[:], lw1[:], op=OP.mult)
        en2 = sb.tile([128, NSUB], F32, tag="en2")
        nc.vector.tensor_tensor(en2[:], w2[:], lw2[:], op=OP.mult)
        entn = sb.tile([128, NSUB], F32, tag="entn")
        nc.vector.tensor_tensor(entn[:], en1[:], en2[:], op=OP.add)

        if _SEC < 6:
            nc.gpsimd.dma_start(Ov[m][:, :, 0:582], stage[:, :, 0:582])
            continue
        # ================= experts + routed sum =================
        Mw = big.tile([128, NSUB, NSC, H], F16, tag="Mw")
        hrps = psB.tile([128, NSUB, H], F32, tag="pss")
        for j in range(NSUB):
            exps = psA.tile([128, NSC * H], F32, tag="big4k")
            nc.tensor.matmul(exps[:, 0:512], stk[:, bass.ts(j, 128)],
                             P["W_ex"][:, 0:512], start=True, stop=True)
            nc.tensor.matmul(exps[:, 512:1024], stk[:, bass.ts(j, 128)],
                             P["W_ex"][:, 512:1024], start=True, stop=True)
            nc.vector.scalar_tensor_tensor(
                Mw[:, j, :, :],
                exps[:].rearrange("p (s d) -> p s d", s=NSC),
                0.0,
                _fv(stage[:, j, OC_SW:OC_SW + NSC], [[1, NSC], [0, H]]),
                op0=OP.max, op1=OP.mult)
            for s in range(NSC):
                nc.tensor.matmul(hrps[:, j, :], I16[:], Mw[:, j, s, :],
                                 start=(s == 0), stop=(s == NSC - 1))
        hr16 = sb.tile([128, NSUB, H], F16, tag="hr16")
        nc.scalar.copy(hr16[:], hrps[:])
        hTrps = psB.tile([H, MEGA], F16, tag="pss")
        for j in range(NSUB):
            nc.tensor.transpose(hTrps[:, bass.ts(j, 128)], hr16[:, j, :],
                                I16[:])
        hrT = big.tile([H, MEGA], F16, tag="hrT")
        nc.vector.tensor_copy(hrT[:], hTrps[:])

        if _SEC < 7:
            nc.gpsimd.dma_start(Ov[m][:, :, 0:582], stage[:, :, 0:582])
            continue
        # ================= metacognition =================
        mcps = psB.tile([128, NSUB, MD], F32, tag="pss")
        for j in range(NSUB):
            nc.tensor.matmul(mcps[:, j, :], xT2[:, bass.ts(j, 128)],
                             P["W_mc"][:], start=True, stop=False)
            nc.tensor.matmul(mcps[:, j, :], onesr[:, bass.ts(j, 128)],
                             P["b_mc"][0:1, :], start=False, stop=True)
        mcacc = None
        for wname, val in (("Wpe", pe), ("Wconf", conf), ("Went", entn)):
            tt = sb.tile([128, NSUB, MD], F16, tag="mctt_" + wname)
            nc.vector.tensor_tensor(
                tt[:],
                _fv(P[wname][:], [[0, NSUB], [1, MD]]),
                _fv(val[:], [[1, NSUB], [0, MD]]),
                op=OP.mult)
            nxt = sb.tile([128, NSUB, MD], F16, tag="mcs_" + wname)
            nc.vector.tensor_tensor(
                nxt[:], mcps[:] if mcacc is None else mcacc[:], tt[:],
                op=OP.add)
            mcacc = nxt
        smn16 = sb.tile([128, NSUB, MD], F16, tag="smn16")
        nc.scalar.activation(smn16[:], mcacc[:], AF.Tanh)
        nc.scalar.copy(stage[:, :, OC_SMN:OC_SMN + MD], smn16[:])
        smTps = psB.tile([MD, MEGA], F16, tag="pss")
        for j in range(NSUB):
            nc.tensor.transpose(smTps[:, bass.ts(j, 128)], smn16[:, j, :],
                                I16[:])
        smT = sb.tile([MD, MEGA], F16, tag="smT")
        nc.vector.tensor_copy(smT[:], smTps[:])

        if _SEC < 8:
            nc.gpsimd.dma_start(Ov[m][:, :, 0:582], stage[:, :, 0:582])
            continue
        # ================= dA + interp =================
        daps = psB.tile([128, NSUB, 4], F32, tag="pss")
        for j in range(NSUB):
            nc.tensor.matmul(daps[:, j, :], smT[:, bass.ts(j, 128)],
                             P["W_dAip"][:], start=True, stop=False)
            nc.tensor.matmul(daps[:, j, :], onesr[:, bass.ts(j, 128)],
                             P["b_dAip"][0:1, :], start=False, stop=True)
        dAe = sb.tile([128, NSUB], F32, tag="dAe")
        nc.scalar.activation(dAe[:], daps[:, :, 0], AF.Exp)
        dA = sb.tile([128, NSUB], F32, tag="dA")
        nc.scalar.activation(dA[:], dAe[:], AF.Ln, bias=1.0)
        ei = sb.tile([128, NSUB, 3], F16, tag="ei")
        nc.scalar.activation(ei[:], daps[:, :, 1:4], AF.Exp)
        si = sb.tile([128, NSUB], F32, tag="si")
        nc.vector.tensor_reduce(si[:], ei[:], axis=AX.X, op=OP.add)
        ri = sb.tile([128, NSUB], F32, tag="ri")
        nc.vector.reciprocal(ri[:], si[:])
        nc.vector.tensor_tensor(stage[:, :, OC_IP:OC_IP + 3], ei[:],
                                _fv(ri[:], [[1, NSUB], [0, 3]]),
                                op=OP.mult)

        if _SEC < 9:
            nc.gpsimd.dma_start(Ov[m][:, :, 0:582], stage[:, :, 0:582])
            continue
        # ================= SSM state =================
        darg = sb.tile([128, NSUB, SD], F16, tag="darg")
        nc.vector.tensor_tensor(
            darg[:],
            _fv(P["negspA"][:], [[0, NSUB], [1, SD]]),
            _fv(dA[:], [[1, NSUB], [0, SD]]),
            op=OP.mult)
        decay = sb.tile([128, NSUB, SD], F16, tag="decay")
        nc.scalar.activation(decay[:], darg[:], AF.Exp)

        mbps = psA.tile([128, NSUB, 256], F32, tag="big4k")
        for j in range(NSUB):
            nc.tensor.matmul(mbps[:, j, 0:160], hrT[:, bass.ts(j, 128)],
                             P["W_mbqv"][:], start=True, stop=False)
            nc.tensor.matmul(mbps[:, j, 0:160], onesr[:, bass.ts(j, 128)],
                             P["b_mbqv"][0:1, :], start=False, stop=True)
        dso = sb.tile([128, NSUB, SD], F16, tag="dso")
        nc.vector.tensor_tensor(dso[:], decay[:], stag[:, :, IC_SO:IC_SO + SD],
                                op=OP.mult)
        nc.vector.tensor_tensor(stage[:, :, OC_SON:OC_SON + SD], dso[:],
                                mbps[:, :, 0:SD], op=OP.add)
        soTps = psB.tile([SD, MEGA], F16, tag="pss")
        for j in range(NSUB):
            nc.tensor.transpose(soTps[:, bass.ts(j, 128)],
                                stage[:, j, OC_SON:OC_SON + SD], I16[:])
        soT = sb.tile([SD, MEGA], F16, tag="soT")
        nc.vector.tensor_copy(soT[:], soTps[:])

        yc1ps = psB.tile([128, NSUB, 96], F32, tag="pss")
        for j in range(NSUB):
            nc.tensor.matmul(yc1ps[:, j, :], soT[:, bass.ts(j, 128)],
                             P["W_yc1"][:], start=True, stop=False)
            nc.tensor.matmul(yc1ps[:, j, :], onesr[:, bass.ts(j, 128)],
                             P["b_yc1"][0:1, :], start=False, stop=True)
        nc.scalar.copy(stage[:, :, OC_Y:OC_Y + H], yc1ps[:, :, 0:64])
        nc.scalar.copy(stage[:, :, OC_C1:OC_C1 + SD], yc1ps[:, :, 64:96])

        if _SEC < 10:
            nc.gpsimd.dma_start(Ov[m][:, :, 0:582], stage[:, :, 0:582])
            continue
        # ================= slot attention =================
        q16 = sb.tile([128, NSUB, H], F16, tag="q16")
        nc.scalar.copy(q16[:], mbps[:, :, 32:96])
        wv16 = sb.tile([128, NSUB, H], F16, tag="wv16")
        nc.scalar.copy(wv16[:], mbps[:, :, 96:160])
        slots_v = stag[:, :, IC_SL:IC_SL + NSLOT * H]
        prod = big.tile([128, NSUB, NSLOT, H], F16, tag="prod")
        nc.vector.tensor_tensor(
            prod[:],
            slots_v.rearrange("p j (n d) -> p j n d", n=NSLOT),
            _fv(q16[:], [[H, NSUB], [0, NSLOT], [1, H]]),
            op=OP.mult)
        att = sb.tile([128, NSUB, NSLOT], F32, tag="att")
        nc.vector.tensor_reduce(att[:], prod[:], axis=AX.X, op=OP.add)
        ea = sb.tile([128, NSUB, NSLOT], F16, tag="ea")
        nc.scalar.activation(ea[:], att[:], AF.Exp)
        sa = sb.tile([128, NSUB], F32, tag="sa")
        nc.vector.tensor_reduce(sa[:], ea[:], axis=AX.X, op=OP.add)
        ra = sb.tile([128, NSUB], F32, tag="ra")
        nc.vector.reciprocal(ra[:], sa[:])
        attn = sb.tile([128, NSUB, NSLOT], F16, tag="attn")
        nc.vector.tensor_tensor(attn[:], ea[:],
                                _fv(ra[:], [[1, NSUB], [0, NSLOT]]),
                                op=OP.mult)
        m2w = big.tile([128, NSUB, NSLOT, H], F16, tag="m2w")
        nc.vector.tensor_tensor(
            m2w[:],
            slots_v.rearrange("p j (n d) -> p j n d", n=NSLOT),
            _fv(attn[:], [[NSLOT, NSUB], [1, NSLOT], [0, H]]),
            op=OP.mult)
        srps = psB.tile([128, NSUB, H], F32, tag="pss")
        for j in range(NSUB):
            for n in range(NSLOT):
                nc.tensor.matmul(srps[:, j, :], I16[:], m2w[:, j, n, :],
                                 start=(n == 0), stop=(n == NSLOT - 1))
        sr16 = sb.tile([128, NSUB, H], F16, tag="sr16")
        nc.scalar.copy(sr16[:], srps[:])
        srTps = psB.tile([H, MEGA], F16, tag="pss")
        for j in range(NSUB):
            nc.tensor.transpose(srTps[:, bass.ts(j, 128)], sr16[:, j, :],
                                I16[:])
        srT = sb.tile([H, MEGA], F16, tag="srT")
        nc.vector.tensor_copy(srT[:], srTps[:])
        # slots_new = slots + attn (x) wv
        aw = big.tile([128, NSUB, NSLOT, H], F16, tag="aw")
        nc.vector.tensor_tensor(
            aw[:],
            _fv(wv16[:], [[H, NSUB], [0, NSLOT], [1, H]]),
            _fv(attn[:], [[NSLOT, NSUB], [1, NSLOT], [0, H]]),
            op=OP.mult)
        nc.vector.tensor_tensor(
            stage[:, :, OC_SLN:OC_SLN + NSLOT * H]
                 .rearrange("p j (n d) -> p j n d", n=NSLOT),
            aw[:],
            slots_v.rearrange("p j (n d) -> p j n d", n=NSLOT),
            op=OP.add)

        if _SEC < 11:
            nc.gpsimd.dma_start(Ov[m][:, :, 0:582], stage[:, :, 0:582])
            continue
        # ================= c2 / policy / value =================
        c2ps = psB.tile([128, NSUB, 64], F32, tag="pss")
        for j in range(NSUB):
            nc.tensor.matmul(c2ps[:, j, :], soT[:, bass.ts(j, 128)],
                             P["W_c2a"][:], start=True, stop=False)
            nc.tensor.matmul(c2ps[:, j, :], srT[:, bass.ts(j, 128)],
                             P["W_c2b"][:], start=False, stop=False)
            nc.tensor.matmul(c2ps[:, j, :], onesr[:, bass.ts(j, 128)],
                             P["b_c2"][0:1, :], start=False, stop=True)
        nc.scalar.copy(stage[:, :, OC_C2:OC_C2 + 64], c2ps[:])
        c2Tps = psB.tile([64, MEGA], F16, tag="pss")
        for j in range(NSUB):
            nc.tensor.transpose(c2Tps[:, bass.ts(j, 128)],
                                stage[:, j, OC_C2:OC_C2 + 64], I16[:])
        c2T = sb.tile([64, MEGA], F16, tag="c2T")
        nc.vector.tensor_copy(c2T[:], c2Tps[:])
        pvps = psB.tile([128, NSUB, 7], F32, tag="pss")
        for j in range(NSUB):
            nc.tensor.matmul(pvps[:, j, :], c2T[:, bass.ts(j, 128)],
                             P["W_piv"][:], start=True, stop=False)
            nc.tensor.matmul(pvps[:, j, :], onesr[:, bass.ts(j, 128)],
                             P["b_piv"][0:1, :], start=False, stop=True)
        ep = sb.tile([128, NSUB, NA], F16, tag="ep")
        nc.scalar.activation(ep[:], pvps[:, :, 0:NA], AF.Exp)
        sp = sb.tile([128, NSUB], F32, tag="sp")
        nc.vector.tensor_reduce(sp[:], ep[:], axis=AX.X, op=OP.add)
        rp = sb.tile([128, NSUB], F32, tag="rp")
        nc.vector.reciprocal(rp[:], sp[:])
        nc.vector.tensor_tensor(stage[:, :, OC_AP:OC_AP + NA], ep[:],
                                _fv(rp[:], [[1, NSUB], [0, NA]]),
                                op=OP.mult)
        nc.scalar.copy(stage[:, :, OC_V:OC_V + 1], pvps[:, :, 6:7])

        if _SEC < 12:
            nc.gpsimd.dma_start(Ov[m][:, :, 0:582], stage[:, :, 0:582])
            continue
        # ================= c0 / causal vars =================
        ccps = psB.tile([128, NSUB, 20], F32, tag="pss")
        for j in range(NSUB):
            nc.tensor.matmul(ccps[:, j, :], stk[0:64, bass.ts(j, 128)],
                             P["W_c0cg"][:], start=True, stop=False)
            nc.tensor.matmul(ccps[:, j, :], onesr[:, bass.ts(j, 128)],
                             P["b_c0cg"][0:1, :], start=False, stop=True)
        nc.scalar.copy(stage[:, :, OC_C0:OC_C0 + 16], ccps[:, :, 0:16])
        nc.scalar.activation(stage[:, :, OC_VAR:OC_VAR + NVAR],
                             ccps[:, :, 16:20], AF.Sigmoid)

        # ================= store =================
        nc.gpsimd.dma_start(Ov[m][:, :, 0:582], stage[:, :, 0:582])  # cast f16->f32

    ctx.close()


# ---------------------------------------------------------------------------
_CACHE = {}
LAST_EXEC_NS = None


def kernel(obs, prev_h, prev_action_oh, S_obj, S_meta, slots, ext_reward,
           params, _trace=False):
    obs = _f32(obs); prev_h = _f32(prev_h); pa = _f32(prev_action_oh)
    S_obj = _f32(S_obj); S_meta = _f32(S_meta)
    slots_in = _f32(slots); ext = _f32(ext_reward)

    X = np.concatenate(
        [obs, prev_h, pa, S_obj, S_meta,
         slots_in.reshape(B, NSLOT * H), ext[:, None]], axis=1)
    assert X.shape == (B, IN_W)

    pt = prep_params(params)
    specs = {k: (v.shape, F16 if v.dtype == np.float16 else F32)
             for k, v in pt.items()}

    key = "kernel"
    if key not in _CACHE:
        _CACHE[key] = build_kernel(specs)
    nc = _CACHE[key]

    in_maps = []
    for c in range(NCORES):
        im = {"X": np.ascontiguousarray(X[c * BC:(c + 1) * BC]),
              "EXT": np.ascontiguousarray(ext[c * BC:(c + 1) * BC])}
        for k, v in pt.items():
            im["p_" + k] = v
        in_maps.append(im)

    kw = {}
    if _trace:
        kw = dict(trace=True)
    res = run_bass_kernel_spmd(nc, in_maps, core_ids=list(range(NCORES)),
                               **kw)
    global LAST_EXEC_NS
    LAST_EXEC_NS = res.exec_time_ns
    OUT = np.concatenate([res.results[c]["OUT"] for c in range(NCORES)],
                         axis=0)

    def col(c0, w):
        return np.ascontiguousarray(OUT[:, c0:c0 + w], dtype=np.float32)

    action_probs = col(OC_AP, NA)
    value = col(OC_V, 1)[:, 0]
    goal = col(OC_GOAL, GD)
    scene_weights = col(OC_SW, NSC)
    interp = col(OC_IP, 3)
    var_probs = col(OC_VAR, NVAR)
    c0 = col(OC_C0, 16)
    c1 = col(OC_C1, SD)
    c2 = col(OC_C2, 64)
    h = col(OC_H, H)
    y = col(OC_Y, H)
    S_obj_new = col(OC_SON, SD)
    S_meta_new = col(OC_SMN, MD)
    slots_new = col(OC_SLN, NSLOT * H).reshape(B, NSLOT, H)
    return (action_probs, value, goal, scene_weights, interp, var_probs,
            c0, c1, c2, h, y, S_obj_new, S_meta_new, slots_new)


def kernel_traced(*args, **kwargs):
    return kernel(*args, **kwargs, _trace=True)


# revision 31
# speedup vs baseline: 1.0509x; 1.0509x over previous
"""Trainium2 Bass kernel for nn_CSMv2Agent (CSMv2 agent step), 8-core data parallel.

Host side: packs inputs into one fused [B, 431] fp32 tensor + packed fp16
parameter tensors, shards batch across 8 NeuronCores, runs one SPMD NEFF,
unpacks the fused [B, 584] output into the reference's 14-tuple.

Device side (per core, B_c = 4096, 8 mega-tiles of 512 samples):
  - fp16 on-chip compute (PSUM accumulation fp32), feature-major lhsT tiles
    produced by PE transposes, batch-major heads, identity-matmul PSUM
    accumulation for the 16-expert routing reduction.
"""

import numpy as np

import concourse.bass as bass
import concourse.tile as tile
from concourse import bacc, mybir
from concourse.bass_utils import run_bass_kernel_spmd

F32 = mybir.dt.float32
F16 = mybir.dt.float16
AF = mybir.ActivationFunctionType
OP = mybir.AluOpType
AX = mybir.AxisListType

# Problem constants
B, OBS, H, NA = 32768, 56, 64, 6
NSC, TOPK, SD, MD, GD, NSLOT, NVAR = 16, 2, 32, 16, 8, 4, 4
NCORES = 8
BC = B // NCORES            # 4096 per core
MEGA = 512                  # samples per mega-tile
NMEGA = BC // MEGA          # 8
NSUB = MEGA // 128          # 4

# Fused input columns (fp16): obs | prev_h | pa | ONE | S_obj | S_meta | ONE | slots
IC_OBS, IC_PH, IC_PA, IC_ONE1 = 0, 56, 120, 126
IC_SO, IC_SM, IC_ONE2, IC_SL = 127, 159, 175, 176
IN_W = 432

# Fused output columns
OC_AP, OC_V, OC_GOAL, OC_SW = 0, 6, 7, 15
OC_IP, OC_VAR, OC_C0, OC_C1, OC_C2 = 31, 34, 38, 54, 86
OC_H, OC_Y, OC_SON, OC_SMN, OC_SLN = 150, 214, 278, 310, 326
OUT_W = 584  # 582 used + 2 pad


def _f16(x):
    return np.ascontiguousarray(np.asarray(x), dtype=np.float16)


def _f32(x):
    return np.ascontiguousarray(np.asarray(x), dtype=np.float32)


def prep_params(p):
    """Host-side packing of the parameter pytree into device tensors."""
    t = {}
    t["W_enc"] = _f16(p["enc_W"])                       # [56,64]
    t["b_enc"] = _f32(np.asarray(p["enc_b"]).reshape(64, 1))
    t["W_wm"] = _f16(np.concatenate(
        [np.asarray(p["wm_W"]), np.asarray(p["wm_b"]).reshape(1, 64)]))  # [71,64]
    mgW = np.asarray(p["mg_W"]); mgb = np.asarray(p["mg_b"]).reshape(1, 8)
    t["W_mg"] = _f16(np.concatenate([mgW[0:64], mgb]))   # [65,8] w/ bias row
    t["Wcap"] = _f32(np.asarray(p["mg_W"])[65])         # [8] fp32: logits path
    t["Wext"] = _f32(np.asarray(p["mg_W"])[66])         # [8] fp32: logits path
    rtW = np.asarray(p["rt_W"]); rtb = np.asarray(p["rt_b"]).reshape(1, 16)
    t["W_rt"] = _f16(np.concatenate([rtW[0:64], rtb]))   # [65,16] w/ bias row
    t["W_rtg"] = _f16(rtW[64:72])                        # [8,16]
    exw = np.asarray(p["ex_W"])                          # [16,64,64]
    exb = np.asarray(p["ex_b"])                          # [16,64]
    wex = np.concatenate(
        [exw.transpose(1, 0, 2).reshape(64, NSC * 64),
         exb.reshape(1, NSC * 64)], axis=0)              # [65,1024]
    t["W_ex"] = _f16(wex)
    t["W_mc"] = _f16(np.concatenate(
        [np.asarray(p["mc_in_W"])[0:32], np.asarray(p["mc_rec_W"]),
         np.asarray(p["mc_in_b"]).reshape(1, 16)], axis=0))  # [49,16] w/ bias
    t["Wpe"] = _f16(np.asarray(p["mc_in_W"])[32] / 64.0)  # [16] /H folded
    t["Wconf"] = _f16(-np.asarray(p["mc_in_W"])[33])     # [16] negated
    t["Went"] = _f16(-np.asarray(p["mc_in_W"])[34])      # [16] negated
    t["W_dAip"] = _f16(np.concatenate([
        np.concatenate([np.asarray(p["dA_W"]), np.asarray(p["ip_W"])], axis=1),
        np.concatenate([np.asarray(p["dA_b"]), np.asarray(p["ip_b"])]).reshape(1, 4)
    ]))  # [17,4] w/ bias row
    t["negspA"] = _f16(-np.log1p(np.exp(np.asarray(p["A"], np.float64))))  # [32]
    wmb = np.concatenate(
        [np.asarray(p["mB_W"]), np.asarray(p["Wq"]) / np.sqrt(H),
         np.asarray(p["Wv"])], axis=1)                   # [64,160]
    bmb = np.concatenate(
        [np.asarray(p["mB_b"]), np.zeros(128, np.float32)]).reshape(1, 160)
    t["W_mbqv"] = _f16(np.concatenate([wmb, bmb]))       # [65,160] w/ bias
    t["W_yc1"] = _f16(np.concatenate([
        np.concatenate([np.asarray(p["mC_W"]), np.asarray(p["c1_W"])], axis=1),
        np.concatenate([np.asarray(p["mC_b"]), np.asarray(p["c1_b"])]).reshape(1, 96)
    ]))  # [33,96] w/ bias row
    t["W_c2a"] = _f16(np.concatenate(
        [np.asarray(p["c2_W"])[0:32], np.asarray(p["c2_b"]).reshape(1, 64)]))  # [33,64]
    t["W_c2b"] = _f16(np.asarray(p["c2_W"])[32:96])      # [64,64]
    t["W_piv"] = _f16(np.concatenate([
        np.concatenate([np.asarray(p["pi_W"]), np.asarray(p["v_W"])], axis=1),
        np.concatenate([np.asarray(p["pi_b"]), np.asarray(p["v_b"])]).reshape(1, 7)
    ]))  # [65,7] w/ bias row
    t["W_c0cg"] = _f16(np.concatenate([
        np.concatenate([np.asarray(p["c0_W"]), np.asarray(p["cg_W"])], axis=1),
        np.concatenate([np.asarray(p["c0_b"]), np.asarray(p["cg_b"])]).reshape(1, 20)
    ]))  # [65,20] w/ bias row
    def _res(w):
        w = np.asarray(w, np.float32)
        return _f16(w.astype(np.float64) - w.astype(np.float16).astype(np.float64))
    t["dW_enc"] = _res(p["enc_W"])
    t["dW_mg"] = _res(np.concatenate([mgW[0:64], mgb]))      # [65,8]
    t["dW_rt"] = _res(np.concatenate([rtW[0:64], rtb]))      # [65,16]
    t["dW_rtg"] = _res(rtW[64:72])
    t["ident"] = _f16(np.eye(128))                       # [128,128]
    t["ones"] = _f16(np.ones((1, MEGA)))                 # [1,512]
    return t


PARAM_SPECS = None  # filled by prep (name -> (shape, dtype))


def _fv(ap2d, dims):
    """Replace the free dims of a (partition, free...) AP with explicit
    [step, count] pairs (element units). Keeps partition dim + offset."""
    return bass.AP(tensor=ap2d.tensor, offset=ap2d.offset,
                   ap=[list(ap2d.ap[0])] + [list(d) for d in dims])


def build_kernel(param_specs):
    nc = bacc.Bacc("TRN2", target_bir_lowering=False, debug=False,
                   num_devices=NCORES)

    dX = nc.dram_tensor("X", [BC, IN_W], F16, kind="ExternalInput").ap()
    dB = nc.dram_tensor("OBS32", [BC, OBS], F32, kind="ExternalInput").ap()
    dE = nc.dram_tensor("EXT", [BC], F32, kind="ExternalInput").ap()
    dO = nc.dram_tensor("OUT", [BC, OUT_W], F16, kind="ExternalOutput").ap()
    dP = {}
    for name, (shape, dt) in param_specs.items():
        dP[name] = nc.dram_tensor("p_" + name, list(shape), dt,
                                  kind="ExternalInput").ap()

    with tile.TileContext(nc) as tc:
        _body(nc, tc, dX, dB, dE, dO, dP)
    nc.compile()
    return nc


def _body(nc, tc, dX, dB, dE, dO, dP):
    import contextlib
    ctx = contextlib.ExitStack()
    cp = ctx.enter_context(tc.tile_pool(name="const", bufs=1))
    io = ctx.enter_context(tc.tile_pool(name="io", bufs=3))
    sb = ctx.enter_context(tc.tile_pool(name="work", bufs=3))
    big = ctx.enter_context(tc.tile_pool(name="big", bufs=3))
    psA = ctx.enter_context(tc.tile_pool(name="psA", bufs=2, space="PSUM"))
    psB = ctx.enter_context(tc.tile_pool(name="psB", bufs=4, space="PSUM"))

    # ---- load params once ----
    P = {}
    for name, ap in dP.items():
        shp = list(ap.shape)
        if len(shp) == 1:
            # row-vector params broadcast to [128, n] tiles
            tl = cp.tile([128, shp[0]], ap.dtype, tag="par_" + name)
            nc.sync.dma_start(tl, bass.AP(tensor=ap.tensor, offset=0,
                                          ap=[[0, 128], [1, shp[0]]]))
        else:
            tl = cp.tile(shp, ap.dtype, tag="par_" + name)
            nc.sync.dma_start(tl, ap)
        P[name] = tl
    I16 = P["ident"]
    onesr = cp.tile([1, MEGA], F16)
    nc.sync.dma_start(onesr, dP["ones"])
    eps8 = cp.tile([128, 1], F32)
    nc.vector.memset(eps8, 1e-8)
    i32 = cp.tile([1, 1], F32)
    nc.vector.memset(i32, 1.0)
    cp_i32full = cp.tile([128, 128], F32)
    nc.gpsimd.memset(cp_i32full[:], 0.0)
    from concourse.masks import make_identity as _mkid
    _mkid(nc, cp_i32full[:], nomemset=True)

    Xv = dX.rearrange("(m j p) c -> m p j c", p=128, j=NSUB)
    Bv = dB.rearrange("(m j p) c -> m p j c", p=128, j=NSUB)
    Ev = dE.rearrange("(m t) -> m t", t=MEGA)
    Ov = dO.rearrange("(m j p) c -> m p j c", p=128, j=NSUB)

    import os as _os
    _nm = int(_os.environ.get("KERNEL_NMEGA", NMEGA))
    _SEC = int(_os.environ.get("KERNEL_SEC", 99))
    for m in range(_nm):
        # ================= loads =================
        stag = io.tile([128, NSUB, IN_W], F16, tag="stag")
        nc.sync.dma_start(stag[:], Xv[m])
        extr = io.tile([1, MEGA], F32, tag="extr")
        nc.sync.dma_start(extr[:], Ev[m][None, :])
        stage = io.tile([128, NSUB, OUT_W], F16, tag="stage")
        if _SEC < 99:
            nc.vector.memset(stage[:], 0.0)

        # ================= input transposes =================
        obs32 = io.tile([128, NSUB, OBS], F32, tag="obs32")
        nc.sync.dma_start(obs32[:], Bv[m])
        i32f = cp_i32full
        t1ps = psB.tile([56, MEGA], F32, tag="pss")
        t1bps = psB.tile([71, MEGA], F16, tag="pss")
        t2ps = psB.tile([49, MEGA], F16, tag="pss")
        for j in range(NSUB):
            nc.tensor.transpose(t1ps[:, bass.ts(j, 128)],
                                obs32[:, j, :], i32f[:])
            nc.tensor.transpose(t1bps[:, bass.ts(j, 128)],
                                stag[:, j, 56:127], I16[:])
            nc.tensor.transpose(t2ps[:, bass.ts(j, 128)],
                                stag[:, j, 127:176], I16[:])
        obsT = big.tile([56, MEGA], F16, tag="obsT")
        nc.scalar.copy(obsT[:], t1ps[:])
        dobsT = big.tile([56, MEGA], F16, tag="dobsT")
        nc.vector.tensor_tensor(dobsT[:], t1ps[:], obsT[:], op=OP.subtract)
        phpaT = big.tile([71, MEGA], F16, tag="phpaT")
        nc.scalar.copy(phpaT[:], t1bps[:])
        xT2 = big.tile([49, MEGA], F16, tag="xT2")
        nc.vector.tensor_copy(xT2[:], t2ps[:])

        if _SEC < 1:
            nc.gpsimd.dma_start(Ov[m][:, :, 0:582], stage[:, :, 0:582])
            continue
        # ================= encoder (feature-major) =================
        hTps = psB.tile([64, MEGA], F32, tag="pss")
        nc.tensor.matmul(hTps[:], P["W_enc"][:], obsT[:],
                         start=True, stop=False)
        nc.tensor.matmul(hTps[:], P["W_enc"][:], dobsT[:],
                         start=False, stop=False)
        nc.tensor.matmul(hTps[:], P["dW_enc"][:], obsT[:],
                         start=False, stop=True)
        hT32 = big.tile([64, MEGA], F32, tag="hT32")
        nc.scalar.activation(hT32[:], hTps[:], AF.Tanh,
                             bias=P["b_enc"][:], scale=1.0)
        stk = big.tile([65, MEGA], F16, tag="stk")   # rows 0:64 hT, 64 ones
        nc.vector.tensor_copy(stk[0:64, :], hT32[:])
        dhT = big.tile([64, MEGA], F16, tag="dhT")
        nc.vector.tensor_tensor(dhT[:], hT32[:], stk[0:64, :], op=OP.subtract)
        nc.sync.dma_start(stk[64:65, :], dP["ones"])

        if _SEC < 2:
            nc.gpsimd.dma_start(Ov[m][:, :, 0:582], stage[:, :, 0:582])
            continue
        # ================= h (batch-major) + world model =================
        hbps = psB.tile([128, NSUB, H], F16, tag="pss")
        for j in range(NSUB):
            nc.tensor.transpose(hbps[:, j, :], stk[0:64, bass.ts(j, 128)],
                                I16[0:64, 0:64])
        nc.scalar.copy(stage[:, :, OC_H:OC_H + H], hbps[:])

        predps = psB.tile([128, NSUB, H], F32, tag="pss")
        for j in range(NSUB):
            nc.tensor.matmul(predps[:, j, :], phpaT[:, bass.ts(j, 128)],
                             P["W_wm"][:], start=True, stop=True)
        diff = sb.tile([128, NSUB, H], F16, tag="diff")
        nc.vector.tensor_tensor(diff[:], predps[:],
                                stage[:, :, OC_H:OC_H + H],
                                op=OP.subtract)
        dsq = sb.tile([128, NSUB, H], F16, tag="dsq")
        nc.vector.tensor_tensor(dsq[:], diff[:], diff[:], op=OP.mult)
        pe = sb.tile([128, NSUB], F32, tag="pe")
        nc.vector.tensor_reduce(pe[:], dsq[:], axis=AX.X, op=OP.add)
        # pe here is SUM of squares; /H folded into Wpe (host) and conf below
        # conf'' = min(pe/H,1) - 1  (W row negated on host)
        confa = sb.tile([128, NSUB], F32, tag="confa")
        nc.vector.tensor_scalar(confa[:], pe[:], 1.0 / H, 1.0,
                                op0=OP.mult, op1=OP.min)
        conf = sb.tile([128, NSUB], F32, tag="conf")
        nc.vector.tensor_scalar(conf[:], confa[:], 1.0, None,
                                op0=OP.subtract)

        if _SEC < 3:
            nc.gpsimd.dma_start(Ov[m][:, :, 0:582], stage[:, :, 0:582])
            continue
        # ================= ext / capability gap =================
        extps = psB.tile([128, NSUB], F32, tag="pss")
        for j in range(NSUB):
            nc.tensor.transpose(extps[:, j:j + 1],
                                extr[0:1, bass.ts(j, 128)], i32[:])
        ext16 = sb.tile([128, NSUB], F32, tag="ext16")
        nc.scalar.copy(ext16[:], extps[:])
        capa = sb.tile([128, NSUB], F32, tag="capa")
        nc.vector.tensor_scalar(capa[:], extps[:], 0.0, 0.1,
                                op0=OP.is_ge, op1=OP.mult)
        capb = sb.tile([128, NSUB], F32, tag="capb")
        nc.vector.tensor_scalar(capb[:], extps[:], 0.0, -1.0,
                                op0=OP.min, op1=OP.mult)
        cap16 = sb.tile([128, NSUB], F32, tag="cap16")
        nc.vector.tensor_tensor(cap16[:], capa[:], capb[:], op=OP.add)

        if _SEC < 4:
            nc.gpsimd.dma_start(Ov[m][:, :, 0:582], stage[:, :, 0:582])
            continue
        # ================= meta goal =================
        goalps = psB.tile([128, NSUB, GD], F32, tag="pss")
        for j in range(NSUB):
            nc.tensor.matmul(goalps[:, j, :], stk[:, bass.ts(j, 128)],
                             P["W_mg"][:], start=True, stop=False)
            nc.tensor.matmul(goalps[:, j, :], dhT[:, bass.ts(j, 128)],
                             P["W_mg"][0:64, :], start=False, stop=False)
            nc.tensor.matmul(goalps[:, j, :], stk[:, bass.ts(j, 128)],
                             P["dW_mg"][:], start=False, stop=True)
        tt1 = sb.tile([128, NSUB, GD], F32, tag="gtt1")
        nc.vector.tensor_tensor(
            tt1[:], _fv(P["Wcap"][:], [[0, NSUB], [1, GD]]),
            _fv(cap16[:], [[1, NSUB], [0, GD]]), op=OP.mult)
        tt2 = sb.tile([128, NSUB, GD], F32, tag="gtt2")
        nc.vector.tensor_tensor(
            tt2[:], _fv(P["Wext"][:], [[0, NSUB], [1, GD]]),
            _fv(ext16[:], [[1, NSUB], [0, GD]]), op=OP.mult)
        gg1 = sb.tile([128, NSUB, GD], F32, tag="gg1")
        nc.vector.tensor_tensor(gg1[:], goalps[:], tt1[:], op=OP.add)
        gg2 = sb.tile([128, NSUB, GD], F32, tag="gg2")
        nc.vector.tensor_tensor(gg2[:], gg1[:], tt2[:], op=OP.add)
        goal32 = sb.tile([128, NSUB, GD], F32, tag="goal32")
        nc.scalar.activation(goal32[:], gg2[:], AF.Tanh)
        goal16 = sb.tile([128, NSUB, GD], F16, tag="goal16")
        nc.vector.tensor_copy(goal16[:], goal32[:])
        dgoal = sb.tile([128, NSUB, GD], F16, tag="dgoal")
        nc.vector.tensor_tensor(dgoal[:], goal32[:], goal16[:],
                                op=OP.subtract)
        nc.scalar.copy(stage[:, :, OC_GOAL:OC_GOAL + GD], goal32[:])
        gTps = psB.tile([GD, MEGA], F16, tag="pss")
        gdTps = psB.tile([GD, MEGA], F16, tag="pss")
        for j in range(NSUB):
            nc.tensor.transpose(gTps[:, bass.ts(j, 128)], goal16[:, j, :],
                                I16[:])
            nc.tensor.transpose(gdTps[:, bass.ts(j, 128)], dgoal[:, j, :],
                                I16[:])
        goalT = sb.tile([GD, MEGA], F16, tag="goalT")
        nc.vector.tensor_copy(goalT[:], gTps[:])
        dgoalT = sb.tile([GD, MEGA], F16, tag="dgoalT")
        nc.scalar.copy(dgoalT[:], gdTps[:])

        if _SEC < 5:
            nc.gpsimd.dma_start(Ov[m][:, :, 0:582], stage[:, :, 0:582])
            continue
        # ================= router logits + top-2 =================
        lps = psB.tile([128, NSUB, NSC], F32, tag="pss")
        for j in range(NSUB):
            nc.tensor.matmul(lps[:, j, :], stk[:, bass.ts(j, 128)],
                             P["W_rt"][:], start=True, stop=False)
            nc.tensor.matmul(lps[:, j, :], dhT[:, bass.ts(j, 128)],
                             P["W_rt"][0:64, :], start=False, stop=False)
            nc.tensor.matmul(lps[:, j, :], stk[:, bass.ts(j, 128)],
                             P["dW_rt"][:], start=False, stop=False)
            nc.tensor.matmul(lps[:, j, :], goalT[:, bass.ts(j, 128)],
                             P["W_rtg"][:], start=False, stop=False)
            nc.tensor.matmul(lps[:, j, :], dgoalT[:, bass.ts(j, 128)],
                             P["W_rtg"][:], start=False, stop=False)
            nc.tensor.matmul(lps[:, j, :], goalT[:, bass.ts(j, 128)],
                             P["dW_rtg"][:], start=False, stop=True)
        L0 = sb.tile([128, NSUB, NSC], F32, tag="L0")
        nc.vector.tensor_copy(L0[:], lps[:])
        mx = sb.tile([128, NSUB, 8], F32, tag="mx")
        for j in range(NSUB):
            nc.vector.max(mx[:, j, :], L0[:, j, :])
        rep = sb.tile([128, NSUB, 8], F32, tag="rep")
        nc.vector.memset(rep[:], 1e30)
        nc.vector.tensor_copy(rep[:, :, 0:1], mx[:, :, 0:1])
        L1 = sb.tile([128, NSUB, NSC], F32, tag="L1")
        for j in range(NSUB):
            nc.vector.match_replace(L1[:, j, :], rep[:, j, :], L0[:, j, :],
                                    -1e30)
        rep2 = sb.tile([128, NSUB, 8], F32, tag="rep2")
        nc.vector.memset(rep2[:], 1e30)
        nc.vector.tensor_copy(rep2[:, :, 0:1], mx[:, :, 1:2])
        L2 = sb.tile([128, NSUB, NSC], F32, tag="L2")
        for j in range(NSUB):
            nc.vector.match_replace(L2[:, j, :], rep2[:, j, :], L1[:, j, :],
                                    -1e30)
        m1 = sb.tile([128, NSUB, NSC], F16, tag="m1")
        nc.vector.tensor_tensor(m1[:], L0[:], L1[:], op=OP.not_equal)
        m2 = sb.tile([128, NSUB, NSC], F16, tag="m2")
        nc.vector.tensor_tensor(m2[:], L1[:], L2[:], op=OP.not_equal)
        dv = sb.tile([128, NSUB], F32, tag="dv")
        nc.vector.tensor_tensor(dv[:], mx[:, :, 1], mx[:, :, 0],
                                op=OP.subtract)
        ew = sb.tile([128, NSUB], F32, tag="ew")
        nc.scalar.activation(ew[:], dv[:], AF.Exp)
        zw = sb.tile([128, NSUB], F32, tag="zw")
        nc.vector.tensor_scalar_add(zw[:], ew[:], 1.0)
        w1 = sb.tile([128, NSUB], F32, tag="w1")
        nc.vector.reciprocal(w1[:], zw[:])
        w2 = sb.tile([128, NSUB], F32, tag="w2")
        nc.vector.tensor_scalar(w2[:], w1[:], -1.0, 1.0,
                                op0=OP.mult, op1=OP.add)
        swst = stage[:, :, OC_SW:OC_SW + NSC]
        nc.vector.tensor_tensor(swst, m1[:],
                                _fv(w1[:], [[1, NSUB], [0, NSC]]),
                                op=OP.mult)
        sw2t = sb.tile([128, NSUB, NSC], F16, tag="sw2t")
        nc.vector.tensor_tensor(sw2t[:], m2[:],
                                _fv(w2[:], [[1, NSUB], [0, NSC]]),
                                op=OP.mult)
        nc.vector.tensor_tensor(swst, swst, sw2t[:], op=OP.add)
        # entropy (negated): w1*ln(w1+eps) + w2*ln(w2+eps)
        lw1 = sb.tile([128, NSUB], F32, tag="lw1")
        nc.scalar.activation(lw1[:], w1[:], AF.Ln, bias=eps8[:])
        lw2 = sb.tile([128, NSUB], F32, tag="lw2")
        nc.scalar.activation(lw2[:], w2[:], AF.Ln, bias=eps8[:])
        en1 = sb.tile([128, NSUB], F32, tag="en1")
        nc.vector.tensor_tensor(en1[:], w1[:], lw1[:], op=OP.mult)
        en2 = sb.tile([128, NSUB], F32, tag="en2")
        nc.vector.tensor_tensor(en2[:], w2[:], lw2[:], op=OP.mult)
        entn = sb.tile([128, NSUB], F32, tag="entn")
        nc.vector.tensor_tensor(entn[:], en1[:], en2[:], op=OP.add)

        if _SEC < 6:
            nc.gpsimd.dma_start(Ov[m][:, :, 0:582], stage[:, :, 0:582])
            continue
        # ================= experts + routed sum =================
        Mw = big.tile([128, NSUB, NSC, H], F16, tag="Mw")
        for j in range(NSUB):
            exps = psA.tile([128, NSC * H], F32, tag="big4k")
            nc.tensor.matmul(exps[:, 0:512], stk[:, bass.ts(j, 128)],
                             P["W_ex"][:, 0:512], start=True, stop=True)
            nc.tensor.matmul(exps[:, 512:1024], stk[:, bass.ts(j, 128)],
                             P["W_ex"][:, 512:1024], start=True, stop=True)
            nc.vector.scalar_tensor_tensor(
                Mw[:, j, :, :],
                exps[:].rearrange("p (s d) -> p s d", s=NSC),
                0.0,
                _fv(stage[:, j, OC_SW:OC_SW + NSC], [[1, NSC], [0, H]]),
                op0=OP.max, op1=OP.mult)
        # reduce over 16 scenes: gpsimd level-1, DVE levels 2-4
        l1 = big.tile([128, NSUB, 8, H], F16, tag="l1r")
        nc.gpsimd.tensor_tensor(l1[:], Mw[:, :, 0:8, :], Mw[:, :, 8:16, :],
                                op=OP.add)
        l2 = sb.tile([128, NSUB, 4, H], F16, tag="l2r")
        nc.vector.tensor_tensor(l2[:], l1[:, :, 0:4, :], l1[:, :, 4:8, :],
                                op=OP.add)
        l3 = sb.tile([128, NSUB, 2, H], F16, tag="l3r")
        nc.vector.tensor_tensor(l3[:], l2[:, :, 0:2, :], l2[:, :, 2:4, :],
                                op=OP.add)
        # final level writes the hr staging (with ones col at 64)
        hr16 = sb.tile([128, NSUB, H + 1], F16, tag="hr16")
        nc.vector.memset(hr16[:, :, H:H + 1], 1.0)
        nc.vector.tensor_tensor(hr16[:, :, 0:H], l3[:, :, 0, :],
                                l3[:, :, 1, :], op=OP.add)
        hTrps = psB.tile([H + 1, MEGA], F16, tag="pss")
        for j in range(NSUB):
            nc.tensor.transpose(hTrps[:, bass.ts(j, 128)], hr16[:, j, :],
                                I16[:])
        hrT = big.tile([H + 1, MEGA], F16, tag="hrT")
        nc.vector.tensor_copy(hrT[:], hTrps[:])

        if _SEC < 7:
            nc.gpsimd.dma_start(Ov[m][:, :, 0:582], stage[:, :, 0:582])
            continue
        # ================= metacognition =================
        mcps = psB.tile([128, NSUB, MD], F32, tag="pss")
        for j in range(NSUB):
            nc.tensor.matmul(mcps[:, j, :], xT2[:, bass.ts(j, 128)],
                             P["W_mc"][:], start=True, stop=True)
        mcacc = None
        for wname, val in (("Wpe", pe), ("Wconf", conf), ("Went", entn)):
            tt = sb.tile([128, NSUB, MD], F16, tag="mctt_" + wname)
            nc.vector.tensor_tensor(
                tt[:],
                _fv(P[wname][:], [[0, NSUB], [1, MD]]),
                _fv(val[:], [[1, NSUB], [0, MD]]),
                op=OP.mult)
            nxt = sb.tile([128, NSUB, MD], F16, tag="mcs_" + wname)
            nc.vector.tensor_tensor(
                nxt[:], mcps[:] if mcacc is None else mcacc[:], tt[:],
                op=OP.add)
            mcacc = nxt
        smn16 = sb.tile([128, NSUB, MD + 1], F16, tag="smn16")
        nc.vector.memset(smn16[:, :, MD:MD + 1], 1.0)
        nc.scalar.activation(smn16[:, :, 0:MD], mcacc[:], AF.Tanh)
        nc.scalar.copy(stage[:, :, OC_SMN:OC_SMN + MD], smn16[:, :, 0:MD])
        smTps = psB.tile([MD + 1, MEGA], F16, tag="pss")
        for j in range(NSUB):
            nc.tensor.transpose(smTps[:, bass.ts(j, 128)], smn16[:, j, :],
                                I16[:])
        smT = sb.tile([MD + 1, MEGA], F16, tag="smT")
        nc.vector.tensor_copy(smT[:], smTps[:])

        if _SEC < 8:
            nc.gpsimd.dma_start(Ov[m][:, :, 0:582], stage[:, :, 0:582])
            continue
        # ================= dA + interp =================
        daps = psB.tile([128, NSUB, 4], F32, tag="pss")
        for j in range(NSUB):
            nc.tensor.matmul(daps[:, j, :], smT[:, bass.ts(j, 128)],
                             P["W_dAip"][:], start=True, stop=True)
        dAe = sb.tile([128, NSUB], F32, tag="dAe")
        nc.scalar.activation(dAe[:], daps[:, :, 0], AF.Exp)
        dA = sb.tile([128, NSUB], F32, tag="dA")
        nc.scalar.activation(dA[:], dAe[:], AF.Ln, bias=1.0)
        ei = sb.tile([128, NSUB, 3], F16, tag="ei")
        nc.scalar.activation(ei[:], daps[:, :, 1:4], AF.Exp)
        si = sb.tile([128, NSUB], F32, tag="si")
        nc.vector.tensor_reduce(si[:], ei[:], axis=AX.X, op=OP.add)
        ri = sb.tile([128, NSUB], F32, tag="ri")
        nc.vector.reciprocal(ri[:], si[:])
        nc.vector.tensor_tensor(stage[:, :, OC_IP:OC_IP + 3], ei[:],
                                _fv(ri[:], [[1, NSUB], [0, 3]]),
                                op=OP.mult)

        if _SEC < 9:
            nc.gpsimd.dma_start(Ov[m][:, :, 0:582], stage[:, :, 0:582])
            continue
        # ================= SSM state =================
        darg = sb.tile([128, NSUB, SD], F16, tag="darg")
        nc.vector.tensor_tensor(
            darg[:],
            _fv(P["negspA"][:], [[0, NSUB], [1, SD]]),
            _fv(dA[:], [[1, NSUB], [0, SD]]),
            op=OP.mult)
        decay = sb.tile([128, NSUB, SD], F16, tag="decay")
        nc.scalar.activation(decay[:], darg[:], AF.Exp)

        mbps = psA.tile([128, NSUB, 256], F32, tag="big4k")
        for j in range(NSUB):
            nc.tensor.matmul(mbps[:, j, 0:160], hrT[:, bass.ts(j, 128)],
                             P["W_mbqv"][:], start=True, stop=True)
        dso = sb.tile([128, NSUB, SD], F16, tag="dso")
        nc.vector.tensor_tensor(dso[:], decay[:], stag[:, :, IC_SO:IC_SO + SD],
                                op=OP.mult)
        son = sb.tile([128, NSUB, SD + 1], F16, tag="son")
        nc.vector.memset(son[:, :, SD:SD + 1], 1.0)
        nc.vector.tensor_tensor(son[:, :, 0:SD], dso[:],
                                mbps[:, :, 0:SD], op=OP.add)
        nc.scalar.copy(stage[:, :, OC_SON:OC_SON + SD], son[:, :, 0:SD])
        soTps = psB.tile([SD + 1, MEGA], F16, tag="pss")
        for j in range(NSUB):
            nc.tensor.transpose(soTps[:, bass.ts(j, 128)],
                                son[:, j, :], I16[:])
        soT = sb.tile([SD + 1, MEGA], F16, tag="soT")
        nc.vector.tensor_copy(soT[:], soTps[:])

        yc1ps = psB.tile([128, NSUB, 96], F32, tag="pss")
        for j in range(NSUB):
            nc.tensor.matmul(yc1ps[:, j, :], soT[:, bass.ts(j, 128)],
                             P["W_yc1"][:], start=True, stop=True)
        nc.scalar.copy(stage[:, :, OC_Y:OC_Y + H], yc1ps[:, :, 0:64])
        nc.scalar.copy(stage[:, :, OC_C1:OC_C1 + SD], yc1ps[:, :, 64:96])

        if _SEC < 10:
            nc.gpsimd.dma_start(Ov[m][:, :, 0:582], stage[:, :, 0:582])
            continue
        # ================= slot attention =================
        q16 = sb.tile([128, NSUB, H], F16, tag="q16")
        nc.scalar.copy(q16[:], mbps[:, :, 32:96])
        wv16 = sb.tile([128, NSUB, H], F16, tag="wv16")
        nc.scalar.copy(wv16[:], mbps[:, :, 96:160])
        slots_v = stag[:, :, IC_SL:IC_SL + NSLOT * H]
        prod = big.tile([128, NSUB, NSLOT, H], F16, tag="prod")
        nc.vector.tensor_tensor(
            prod[:],
            slots_v.rearrange("p j (n d) -> p j n d", n=NSLOT),
            _fv(q16[:], [[H, NSUB], [0, NSLOT], [1, H]]),
            op=OP.mult)
        att = sb.tile([128, NSUB, NSLOT], F32, tag="att")
        nc.vector.tensor_reduce(att[:], prod[:], axis=AX.X, op=OP.add)
        ea = sb.tile([128, NSUB, NSLOT], F16, tag="ea")
        nc.scalar.activation(ea[:], att[:], AF.Exp)
        sa = sb.tile([128, NSUB], F32, tag="sa")
        nc.vector.tensor_reduce(sa[:], ea[:], axis=AX.X, op=OP.add)
        ra = sb.tile([128, NSUB], F32, tag="ra")
        nc.vector.reciprocal(ra[:], sa[:])
        attn = sb.tile([128, NSUB, NSLOT], F16, tag="attn")
        nc.vector.tensor_tensor(attn[:], ea[:],
                                _fv(ra[:], [[1, NSUB], [0, NSLOT]]),
                                op=OP.mult)
        m2w = big.tile([128, NSUB, NSLOT, H], F16, tag="m2w")
        nc.vector.tensor_tensor(
            m2w[:],
            slots_v.rearrange("p j (n d) -> p j n d", n=NSLOT),
            _fv(attn[:], [[NSLOT, NSUB], [1, NSLOT], [0, H]]),
            op=OP.mult)
        sl1 = sb.tile([128, NSUB, 2, H], F16, tag="sl1")
        nc.vector.tensor_tensor(sl1[:], m2w[:, :, 0:2, :], m2w[:, :, 2:4, :],
                                op=OP.add)
        sr16 = sb.tile([128, NSUB, H], F16, tag="sr16")
        nc.vector.tensor_tensor(sr16[:], sl1[:, :, 0, :], sl1[:, :, 1, :],
                                op=OP.add)
        srTps = psB.tile([H, MEGA], F16, tag="pss")
        for j in range(NSUB):
            nc.tensor.transpose(srTps[:, bass.ts(j, 128)], sr16[:, j, :],
                                I16[:])
        srT = sb.tile([H, MEGA], F16, tag="srT")
        nc.vector.tensor_copy(srT[:], srTps[:])
        # slots_new = slots + attn (x) wv
        aw = big.tile([128, NSUB, NSLOT, H], F16, tag="aw")
        nc.vector.tensor_tensor(
            aw[:],
            _fv(wv16[:], [[H, NSUB], [0, NSLOT], [1, H]]),
            _fv(attn[:], [[NSLOT, NSUB], [1, NSLOT], [0, H]]),
            op=OP.mult)
        nc.vector.tensor_tensor(
            stage[:, :, OC_SLN:OC_SLN + NSLOT * H]
                 .rearrange("p j (n d) -> p j n d", n=NSLOT),
            aw[:],
            slots_v.rearrange("p j (n d) -> p j n d", n=NSLOT),
            op=OP.add)

        if _SEC < 11:
            nc.gpsimd.dma_start(Ov[m][:, :, 0:582], stage[:, :, 0:582])
            continue
        # ================= c2 / policy / value =================
        c2ps = psB.tile([128, NSUB, 64], F32, tag="pss")
        for j in range(NSUB):
            nc.tensor.matmul(c2ps[:, j, :], soT[:, bass.ts(j, 128)],
                             P["W_c2a"][:], start=True, stop=False)
            nc.tensor.matmul(c2ps[:, j, :], srT[:, bass.ts(j, 128)],
                             P["W_c2b"][:], start=False, stop=True)
        c2s = sb.tile([128, NSUB, 65], F16, tag="c2s")
        nc.vector.memset(c2s[:, :, 64:65], 1.0)
        nc.scalar.copy(c2s[:, :, 0:64], c2ps[:])
        nc.scalar.copy(stage[:, :, OC_C2:OC_C2 + 64], c2s[:, :, 0:64])
        c2Tps = psB.tile([65, MEGA], F16, tag="pss")
        for j in range(NSUB):
            nc.tensor.transpose(c2Tps[:, bass.ts(j, 128)],
                                c2s[:, j, :], I16[:])
        c2T = sb.tile([65, MEGA], F16, tag="c2T")
        nc.vector.tensor_copy(c2T[:], c2Tps[:])
        pvps = psB.tile([128, NSUB, 7], F32, tag="pss")
        for j in range(NSUB):
            nc.tensor.matmul(pvps[:, j, :], c2T[:, bass.ts(j, 128)],
                             P["W_piv"][:], start=True, stop=True)
        ep = sb.tile([128, NSUB, NA], F16, tag="ep")
        nc.scalar.activation(ep[:], pvps[:, :, 0:NA], AF.Exp)
        sp = sb.tile([128, NSUB], F32, tag="sp")
        nc.vector.tensor_reduce(sp[:], ep[:], axis=AX.X, op=OP.add)
        rp = sb.tile([128, NSUB], F32, tag="rp")
        nc.vector.reciprocal(rp[:], sp[:])
        nc.vector.tensor_tensor(stage[:, :, OC_AP:OC_AP + NA], ep[:],
                                _fv(rp[:], [[1, NSUB], [0, NA]]),
                                op=OP.mult)
        nc.scalar.copy(stage[:, :, OC_V:OC_V + 1], pvps[:, :, 6:7])

        if _SEC < 12:
            nc.gpsimd.dma_start(Ov[m][:, :, 0:582], stage[:, :, 0:582])
            continue
        # ================= c0 / causal vars =================
        ccps = psB.tile([128, NSUB, 20], F32, tag="pss")
        for j in range(NSUB):
            nc.tensor.matmul(ccps[:, j, :], stk[:, bass.ts(j, 128)],
                             P["W_c0cg"][:], start=True, stop=True)
        nc.scalar.copy(stage[:, :, OC_C0:OC_C0 + 16], ccps[:, :, 0:16])
        ven = sb.tile([128, NSUB, NVAR], F32, tag="ven")
        nc.scalar.activation(ven[:], ccps[:, :, 16:20], AF.Exp, scale=-1.0)
        vz = sb.tile([128, NSUB, NVAR], F32, tag="vz")
        nc.vector.tensor_scalar_add(vz[:], ven[:], 1.0)
        vr = sb.tile([128, NSUB, NVAR], F32, tag="vr")
        nc.vector.reciprocal(vr[:], vz[:])
        nc.vector.tensor_copy(stage[:, :, OC_VAR:OC_VAR + NVAR], vr[:])

        # ================= store =================
        nc.sync.dma_start(Ov[m][:, :, 0:582], stage[:, :, 0:582])

    ctx.close()


# ---------------------------------------------------------------------------
_CACHE = {}
LAST_EXEC_NS = None


def kernel(obs, prev_h, prev_action_oh, S_obj, S_meta, slots, ext_reward,
           params, _trace=False):
    obs = _f32(obs); ext = _f32(ext_reward)
    one = np.ones((B, 1), np.float16)
    X = np.concatenate(
        [_f16(obs), _f16(prev_h), _f16(prev_action_oh), one,
         _f16(S_obj), _f16(S_meta), one,
         _f16(np.asarray(slots).reshape(B, NSLOT * H))], axis=1)
    assert X.shape == (B, IN_W)

    pt = prep_params(params)
    specs = {k: (v.shape, F16 if v.dtype == np.float16 else F32)
             for k, v in pt.items()}

    key = "kernel"
    if key not in _CACHE:
        _CACHE[key] = build_kernel(specs)
    nc = _CACHE[key]

    in_maps = []
    for c in range(NCORES):
        im = {"X": np.ascontiguousarray(X[c * BC:(c + 1) * BC]),
              "OBS32": np.ascontiguousarray(obs[c * BC:(c + 1) * BC]),
              "EXT": np.ascontiguousarray(ext[c * BC:(c + 1) * BC])}
        for k, v in pt.items():
            im["p_" + k] = v
        in_maps.append(im)

    kw = {}
    if _trace:
        kw = dict(trace=True)
    res = run_bass_kernel_spmd(nc, in_maps, core_ids=list(range(NCORES)),
                               **kw)
    global LAST_EXEC_NS
    LAST_EXEC_NS = res.exec_time_ns
    OUT = np.concatenate([res.results[c]["OUT"] for c in range(NCORES)],
                         axis=0)

    OUT = OUT.astype(np.float32)

    def col(c0, w):
        return np.ascontiguousarray(OUT[:, c0:c0 + w])

    action_probs = col(OC_AP, NA)
    value = col(OC_V, 1)[:, 0]
    goal = col(OC_GOAL, GD)
    scene_weights = col(OC_SW, NSC)
    interp = col(OC_IP, 3)
    var_probs = col(OC_VAR, NVAR)
    c0 = col(OC_C0, 16)
    c1 = col(OC_C1, SD)
    c2 = col(OC_C2, 64)
    h = col(OC_H, H)
    y = col(OC_Y, H)
    S_obj_new = col(OC_SON, SD)
    S_meta_new = col(OC_SMN, MD)
    slots_new = col(OC_SLN, NSLOT * H).reshape(B, NSLOT, H)
    return (action_probs, value, goal, scene_weights, interp, var_probs,
            c0, c1, c2, h, y, S_obj_new, S_meta_new, slots_new)


def kernel_traced(*args, **kwargs):
    return kernel(*args, **kwargs, _trace=True)
